# revision 22
# baseline (speedup 1.0000x reference)
"""Trainium2 Bass kernel for 2-layer GCN (GCNConv -> relu -> GCNConv -> Linear).

Strategy (8 NeuronCores, SPMD), v2:
  - Nodes padded to NPAD=100352 and dealt (serpentine, by degree) into 784
    blocks of 128 slots; 98 blocks per core (edge partition by destination).
  - Layer-1 table is just xd = (x * dinv)[position order] (bf16), uploaded as
    an input: aggregation commutes with the linear transform, so x is
    aggregated first and @W1 happens per destination block afterwards.  No
    on-device table build, no T1 AllGather, no barrier before L1.
  - Aggregation is TRANSPOSED: psum[feat, slot] += gt_chunk.T @ S_chunk where
    S is the 0/1 selection matrix (DVE is_equal) and gt are dma_gather'ed
    message rows.  The dst-side dinv is then a per-block DVE multiply with a
    partition-replicated dinv panel (no PE transpose in the epilogue).
  - Self loops are never gathered: one identity matmul per block on the
    contiguous local tile (xself panel for L1, the SBUF-resident u2panel for
    L2).
  - Gather streams: per (queue, block) cells padded to multiples of 32 with
    cross-core-max lengths (SPMD shares one program); matmul "runs" merge
    contiguous rows within each 128-row gather column (~20 matmuls/block).
    216.5K rows/core/layer vs 250.9K in v1.
  - Layer-2: u2 rows are duplicated to 256B rows (min gather row size) and
    the table is split into 4 BLOCK-RANGE quarters [25,25,25,23].  Each
    quarter is AllGather'ed by its own collective as soon as its last
    block's epilogue lands (explicit cross-engine deps, no barriers), and
    L2 queue j gathers exclusively from quarter j -- so the collectives
    hide under the L1 gather drain and only the last quarter's (~5 MB) is
    exposed.  Queue loads are rebalanced via the quarter sizes.

Perf notes (measured on these cores):
  - dma_gather is descriptor-paced ~8.4 ns per 256B row per SWDGE queue
    (4-queue hard limit); 426K rows/core over both layers -> ~900 us floor,
    measured directly with KB_AGG=0.  Larger dynamic_dma_scratch_size eats
    SBUF and did not help beyond 65536.
  - Deep gather-tile buffering (GBUF=36) absorbs the epilogue/collective
    pipeline jitter; selection-matrix builds are emitted one block ahead so
    the strict-FIFO DVE never blocks the next block's PE matmuls behind a
    stalled psum read.
  - AllGather outputs must be addr_space="Shared" for the fast HBM-HBM path.
  - Removing the collective/gather ordering deps desyncs the mesh
    (NRT_EXEC_UNIT_UNRECOVERABLE): keep the explicit _add_dep_helper edges.
  - Uniform CALL=1024-index gather calls (partial/2048 calls wedged).
"""

import os
import numpy as np
import ml_dtypes

P = 128
NCORES = 8
NQ = 4
IN_C, HID, OUT_C = 128, 128, 64
CALL = int(os.environ.get("KB_CALL", "1024"))


def _set_size(n_nodes, bpc):
    global N, BPC, NBINS, NPAD, SHARD, WIN, PAIRWIN
    N = n_nodes
    BPC = bpc
    NBINS = NCORES * BPC
    NPAD = NBINS * P
    SHARD = BPC * P
    WIN = NPAD // NQ          # L1 gather window (rows)
    PAIRWIN = NPAD // 4       # L2 gather window (pair rows); 2 windows of NPAD//4 pairs
    assert WIN <= 32768 and PAIRWIN <= 32768


_set_size(100000, 98)

QBLK = [25, 25, 25, 23]                    # L2 source-quarter sizes (blocks)
QB0 = [0, 25, 50, 75]

_kernel_cache = {}


def _wrap_idx(st):
    """[C, SLEN] int16 -> [C, 128, SLEN//16] wrapped+replicated."""
    C, SLEN = st.shape
    w = st.reshape(C, SLEN // 16, 16)
    w = np.swapaxes(w, 1, 2)                       # [C, 16, SLEN//16]
    return np.ascontiguousarray(np.tile(w, (1, 8, 1)))


def _build_layer(core, q, blk, idx, dslot):
    """Build gather streams + schedule for one layer.

    core/q/blk/idx/dslot: per-edge arrays (self loops excluded).
    Layout is shared across cores (cross-core max cell lengths); cells are
    packed back-to-back with NO alignment: every matmul consumes a full
    128-row gather column, and rows of the adjacent block are masked to zero
    in the selection matrix via block-parity disambiguation (dstloc value =
    dslot + 128*(block%2), compared against the block's parity window of a
    [P, 256] iota table; pad rows use sentinel 500).

    Returns (sched, gidx, dstloc):
      sched: dict with slen[q], colbase[q], blocks[b] = per-q (c0, c1, need)
      gidx:  list per q of [NCORES, 128, slen//16] int16
      dstloc: [NCORES, 128, TOTCOLS] bf16
    """
    ncell = NCORES * NQ * BPC
    cell = (core * NQ + q) * BPC + blk
    cnt = np.bincount(cell, minlength=ncell).reshape(NCORES, NQ, BPC)
    ulen = cnt.max(axis=0).astype(np.int64)                           # [NQ, BPC]
    start = np.zeros((NQ, BPC), np.int64)
    start[:, 1:] = np.cumsum(ulen, axis=1)[:, :-1]
    qlen = ulen.sum(axis=1)
    slen = (-(-qlen // CALL)) * CALL                                  # [NQ]
    sbase = np.zeros(NQ + 1, np.int64)
    np.cumsum(slen, out=sbase[1:])
    stot = int(sbase[-1])

    # place edges into per-(core, queue) streams
    order = np.argsort(cell, kind="stable")
    ccnt = np.bincount(cell, minlength=ncell)
    cstart = np.zeros(ncell + 1, np.int64)
    np.cumsum(ccnt, out=cstart[1:])
    rank = np.arange(cell.shape[0]) - cstart[cell[order]]
    co, qo, bo = core[order], q[order], blk[order]
    pos = co * stot + sbase[qo] + start[qo, bo] + rank
    idx_flat = np.zeros(NCORES * stot, np.int16)
    dsl_flat = np.full(NCORES * stot, 500.0, np.float32)
    idx_flat[pos] = idx[order].astype(np.int16)
    dsl_flat[pos] = dslot[order] + 128.0 * (bo % 2)

    gidx = []
    for qq in range(NQ):
        st = idx_flat.reshape(NCORES, stot)[:, sbase[qq] : sbase[qq] + slen[qq]]
        gidx.append(_wrap_idx(np.ascontiguousarray(st)))
    colbase = (sbase[:NQ] // P).astype(np.int64)
    totcols = stot // P
    dl = dsl_flat.reshape(NCORES, totcols, P).transpose(0, 2, 1)  # [C, 128, cols]
    dstloc = np.ascontiguousarray(dl).astype(ml_dtypes.bfloat16)

    blocks = []
    for b in range(BPC):
        per_q = []
        for qq in range(NQ):
            o = int(start[qq, b])
            L = int(ulen[qq, b])
            e = o + L
            if L:
                per_q.append((o // P, (e - 1) // P + 1, (e - 1) // CALL))
            else:
                per_q.append((o // P, o // P, -1))
        blocks.append(per_q)
    sched = {
        "slen": [int(s) for s in slen],
        "colbase": [int(c) for c in colbase],
        "totcols": totcols,
        "blocks": blocks,
        "ncalls": [int(s) // CALL for s in slen],
        "cmax": max(
            max((pq[1] - pq[0]) for pq in per_q) for per_q in blocks
        ),
    }
    return sched, gidx, dstloc


def _preprocess(x, edge_index, W1, b1, W2, b2, Wfc, bfc):
    src = np.asarray(edge_index[0], dtype=np.int64)
    dst = np.asarray(edge_index[1], dtype=np.int64)
    deg = (np.bincount(dst, minlength=N) + 1).astype(np.float32)
    dinv = (1.0 / np.sqrt(deg)).astype(np.float32)

    # serpentine deal by degree -> (bin, slot); balances per-block edge counts
    key = np.zeros(NPAD, np.float32)
    key[:N] = deg
    order = np.argsort(-key, kind="stable")
    i = np.arange(NPAD)
    r, c = i // NBINS, i % NBINS
    bins_for_rank = np.where(r % 2 == 0, c, NBINS - 1 - c)
    perm_bin = np.empty(NPAD, np.int64)
    perm_slot = np.empty(NPAD, np.int64)
    perm_bin[order] = bins_for_rank
    perm_slot[order] = r
    perm_pos = perm_bin * P + perm_slot          # node -> position
    pos2node = np.empty(NPAD, np.int64)
    pos2node[perm_pos] = np.arange(NPAD)

    dinv_pad = np.ones(NPAD, np.float32)
    dinv_pad[:N] = dinv
    dinv_pos = dinv_pad[pos2node]                # dinv by position

    # xd table in POSITION order: xd[pos] = x[node] * dinv[node]
    xdn = np.zeros((NPAD, IN_C), np.float32)
    xdn[:N] = np.asarray(x, np.float32) * dinv[:, None]
    xd = xdn[pos2node].astype(ml_dtypes.bfloat16)             # [NPAD, 128]

    ecore = perm_bin[dst] // BPC
    eb = perm_bin[dst] % BPC
    edslot = perm_slot[dst].astype(np.float32)
    spos = perm_pos[src]

    # L1: window = position quarter, idx = position within window
    q1 = spos // WIN
    sched1, gidx1, dstloc1 = _build_layer(ecore, q1, eb, spos % WIN, edslot)
    # L2: queue = source block-range quarter, idx = row in the quarter table
    qb = np.zeros(BPC, np.int64)
    for j in range(1, NQ):
        qb[QB0[j]:] += 1
    c_src = spos // SHARD
    b_src = (spos % SHARD) // P
    s_src = spos % P
    j2 = qb[b_src]
    rows_j = np.asarray([QBLK[j] * P for j in range(NQ)], np.int64)
    idx2 = c_src * rows_j[j2] + (b_src - np.asarray(QB0)[j2]) * P + s_src
    sched2, gidx2, dstloc2 = _build_layer(ecore, j2, eb, idx2, edslot)

    dinv_blk = dinv_pos.reshape(NCORES, BPC, P).transpose(0, 2, 1)  # [C,128,98]
    dinvrep = np.tile(dinv_pos.reshape(NCORES, 1, BPC * P), (1, P, 1)).astype(
        ml_dtypes.bfloat16
    )                                                               # [C,128,BPC*128]

    common = {
        "W1": np.asarray(W1, np.float32).astype(ml_dtypes.bfloat16),
        "W2": np.asarray(W2, np.float32).astype(ml_dtypes.bfloat16),
        "Wfc": np.asarray(Wfc, np.float32).astype(ml_dtypes.bfloat16),
        "b1c": np.asarray(b1, np.float32).reshape(HID, 1),
        "b2c": np.asarray(b2, np.float32).reshape(OUT_C, 1),
        "bfcr": np.asarray(bfc, np.float32).astype(ml_dtypes.bfloat16).reshape(1, OUT_C),
        "ones1": np.ones((1, P), ml_dtypes.bfloat16),
        "identbf": np.eye(P, dtype=ml_dtypes.bfloat16),
        "iota": np.tile(np.arange(2 * P, dtype=np.float32)[None, :], (P, 1)).astype(
            ml_dtypes.bfloat16
        ),
        "xd": xd,
    }
    in_maps = []
    for cc in range(NCORES):
        m = dict(common)
        m["xself"] = np.ascontiguousarray(xd[cc * SHARD : (cc + 1) * SHARD])
        m["dinv_blk"] = np.ascontiguousarray(dinv_blk[cc])
        m["dinvrep"] = np.ascontiguousarray(dinvrep[cc])
        m["dstloc1"] = np.ascontiguousarray(dstloc1[cc])
        m["dstloc2"] = np.ascontiguousarray(dstloc2[cc])
        for qq in range(NQ):
            m[f"gidx1_{qq}"] = np.ascontiguousarray(gidx1[qq][cc])
            m[f"gidx2_{qq}"] = np.ascontiguousarray(gidx2[qq][cc])
        in_maps.append(m)
    return sched1, sched2, in_maps, perm_pos


def _build(sched1, sched2):
    import concourse.bass as bass  # noqa: F401
    import concourse.mybir as mybir
    import concourse.tile as tile
    from concourse import bacc
    from concourse.bass import _add_dep_helper as _add_dep

    stop_after = os.environ.get("KB_STOP_AFTER", "")   # "", "L1", "CC"
    bar_mask = os.environ.get("KB_BARRIERS", "11")
    bar_mask = "11" if bar_mask == "1" else ("00" if bar_mask == "0" else bar_mask)
    dbg_epi = os.environ.get("KB_EPI", "1") == "1"
    dbg_agg = os.environ.get("KB_AGG", "1") == "1"
    GBUF = int(os.environ.get("KB_GBUF", "36"))
    dt = mybir.dt
    OP = mybir.AluOpType

    nc = bacc.Bacc("TRN2", num_devices=NCORES, target_bir_lowering=False, debug=False,
                   num_swdge_queues=4,
                   dynamic_dma_scratch_size=int(os.environ.get("KB_SCRATCH", "65536")))

    xd = nc.dram_tensor("xd", [NPAD, IN_C], dt.bfloat16, kind="ExternalInput")
    xself = nc.dram_tensor("xself", [SHARD, IN_C], dt.bfloat16, kind="ExternalInput")
    W1 = nc.dram_tensor("W1", [IN_C, HID], dt.bfloat16, kind="ExternalInput")
    W2 = nc.dram_tensor("W2", [HID, OUT_C], dt.bfloat16, kind="ExternalInput")
    Wfc = nc.dram_tensor("Wfc", [OUT_C, OUT_C], dt.bfloat16, kind="ExternalInput")
    b1c = nc.dram_tensor("b1c", [HID, 1], dt.float32, kind="ExternalInput")
    b2c = nc.dram_tensor("b2c", [OUT_C, 1], dt.float32, kind="ExternalInput")
    bfcr = nc.dram_tensor("bfcr", [1, OUT_C], dt.bfloat16, kind="ExternalInput")
    ones1 = nc.dram_tensor("ones1", [1, P], dt.bfloat16, kind="ExternalInput")
    identbf = nc.dram_tensor("identbf", [P, P], dt.bfloat16, kind="ExternalInput")
    iota = nc.dram_tensor("iota", [P, 2 * P], dt.bfloat16, kind="ExternalInput")
    dinv_blk = nc.dram_tensor("dinv_blk", [P, BPC], dt.float32, kind="ExternalInput")
    dinvrep = nc.dram_tensor("dinvrep", [P, BPC * P], dt.bfloat16, kind="ExternalInput")
    dstloc1 = nc.dram_tensor("dstloc1", [P, sched1["totcols"]], dt.bfloat16, kind="ExternalInput")
    dstloc2 = nc.dram_tensor("dstloc2", [P, sched2["totcols"]], dt.bfloat16, kind="ExternalInput")
    gidx1 = [nc.dram_tensor(f"gidx1_{q}", [P, sched1["slen"][q] // 16], dt.int16,
                            kind="ExternalInput") for q in range(NQ)]
    gidx2 = [nc.dram_tensor(f"gidx2_{q}", [P, sched2["slen"][q] // 16], dt.int16,
                            kind="ExternalInput") for q in range(NQ)]
    y = nc.dram_tensor("y", [SHARD, OUT_C], dt.float32, kind="ExternalOutput")

    u2locq = [nc.dram_tensor(f"u2loc_{j}", [QBLK[j] * P, 2 * OUT_C], dt.bfloat16)
              for j in range(NQ)]
    T2q = [nc.dram_tensor(f"T2_{j}", [NCORES * QBLK[j] * P, 2 * OUT_C], dt.bfloat16,
                          addr_space="Shared") for j in range(NQ)]

    def layer_pass(tc, sched, tables, gidx_t, dstloc_t, dinvrep_t, gp, sp, ps, pse,
                   gconst, nfeat, rhs_off, epilogue, self_lhsT,
                   gates=None, post_block=None):
        """One aggregation layer: gathers + selection matmuls + epilogues.

        tables: per-q in_ap for dma_gather; rhs_off: per-q feature offset into
        the gathered 128-elem row; nfeat: features per message; self_lhsT(b):
        [128, nfeat] SBUF AP of the block's self rows; epilogue(b, psum) with
        psum [nfeat, 128] = aggregated messages transposed (no dst dinv yet).
        gates[q]: instruction each queue-q gather must wait for (collective);
        post_block(b): hook after block b's epilogue (emits u2 DMAs+collectives).
        """
        issued = [0] * NQ
        gtiles = {}

        def issue(q):
            call = issued[q]
            gt = gp.tile([P, CALL // P, P], dt.bfloat16, tag="gbuf")
            g = nc.gpsimd.dma_gather(
                out_ap=gt[:],
                in_ap=tables[q],
                idxs_ap=gidx_t[q][:, call * (CALL // 16) : (call + 1) * (CALL // 16)],
                num_idxs=CALL,
                num_idxs_reg=CALL,
                elem_size=P,
                queue_num=q,
                single_packet=os.environ.get("KB_SP", "1") == "1",
            )
            if gates is not None and gates[q] is not None:
                _add_dep(g.ins, gates[q].ins, sync=True,
                         reason="gather waits for quarter allgather")
            gtiles[(q, call)] = gt
            issued[q] = call + 1

        cmax = sched["cmax"]

        def build_s(b):
            per_q = sched["blocks"][b]
            par = b % 2
            stiles = []
            for q in range(NQ):
                c0, c1, _ = per_q[q]
                st = sp.tile([P, cmax, P], dt.bfloat16, tag=f"s{q}")
                if c1 > c0:
                    cb = sched["colbase"][q]
                    nc.vector.tensor_tensor(
                        out=st[:, : c1 - c0, :],
                        in0=dstloc_t[:, cb + c0 : cb + c1].to_broadcast([P, c1 - c0, P]),
                        in1=gconst["iota"][:, par * P : (par + 1) * P]
                        .rearrange("p (a b) -> p a b", a=1)
                        .to_broadcast([P, c1 - c0, P]),
                        op=OP.is_equal,
                    )
                stiles.append(st)
            return stiles

        stiles_next = build_s(0) if dbg_agg else None
        for b in range(BPC):
            per_q = sched["blocks"][b]
            for q in range(NQ):
                need = per_q[q][2]
                while issued[q] <= need and issued[q] < sched["ncalls"][q]:
                    issue(q)
            if not dbg_agg:
                continue
            stiles = stiles_next
            psum = ps.tile([nfeat, P], dt.float32, space="PSUM", tag="aggT")
            nc.tensor.matmul(out=psum[:], lhsT=self_lhsT(b), rhs=gconst["identbf"][:],
                             start=True, stop=False)
            nruns = sum(pq[1] - pq[0] for pq in per_q)
            k = 0
            for q in range(NQ):
                c0, c1, _ = per_q[q]
                for col in range(c0, c1):
                    gt = gtiles[(q, col // (CALL // P))]
                    k += 1
                    nc.tensor.matmul(
                        out=psum[:],
                        lhsT=gt[:, col % (CALL // P),
                                rhs_off[q] : rhs_off[q] + nfeat],
                        rhs=stiles[q][:, col - c0, :],
                        start=False,
                        stop=(k == nruns),
                    )
            if b + 1 < BPC:
                stiles_next = build_s(b + 1)
            if dbg_epi:
                epilogue(b, psum)
            else:
                junk = pse.tile([nfeat, P], dt.float32, tag="junk")
                nc.scalar.copy(out=junk[:], in_=psum[:])
            if post_block is not None:
                post_block(b)

    with tile.TileContext(nc) as tc:
        with tc.tile_pool(name="const", bufs=1) as cp:
            gconst = {}
            for name, t, shape, dtt in [
                ("W1", W1, [IN_C, HID], dt.bfloat16),
                ("W2", W2, [HID, OUT_C], dt.bfloat16),
                ("Wfc", Wfc, [OUT_C, OUT_C], dt.bfloat16),
                ("b1c", b1c, [HID, 1], dt.float32),
                ("b2c", b2c, [OUT_C, 1], dt.float32),
                ("bfcr", bfcr, [1, OUT_C], dt.bfloat16),
                ("ones1", ones1, [1, P], dt.bfloat16),
                ("identbf", identbf, [P, P], dt.bfloat16),
                ("iota", iota, [P, 2 * P], dt.bfloat16),
                ("dinv_blk", dinv_blk, [P, BPC], dt.float32),
                ("dinvrep", dinvrep, [P, BPC * P], dt.bfloat16),
            ]:
                tl = cp.tile(shape, dtt, tag=name)
                nc.sync.dma_start(out=tl[:], in_=t[:])
                gconst[name] = tl

            for _rep in range(int(os.environ.get("KB_REPEAT", "1"))):
                if _rep > 0:
                    tc.strict_bb_all_engine_barrier()
                # ---------------- L1: gather xd, aggregate, u2 ------------------
                u2cm = tc.tile_pool(name="u2p", bufs=1)
                u2pool = u2cm.__enter__()
                u2panel = u2pool.tile([P, BPC * OUT_C], dt.bfloat16, tag="u2panel")
                g1cm = tc.tile_pool(name="g1", bufs=1)
                g1p = g1cm.__enter__()
                gidx1_t = []
                for q in range(NQ):
                    tl = g1p.tile([P, sched1["slen"][q] // 16], dt.int16, tag=f"gi1_{q}")
                    nc.sync.dma_start(out=tl[:], in_=gidx1[q][:])
                    gidx1_t.append(tl)
                tl = g1p.tile([P, sched1["totcols"]], dt.bfloat16, tag="dl1")
                nc.sync.dma_start(out=tl[:], in_=dstloc1[:])
                dstloc1_t = tl
                if not (dbg_epi and dbg_agg):
                    nc.vector.memset(u2panel[:], 0)

                tbl1 = [xd[q * WIN : (q + 1) * WIN, :] for q in range(NQ)]

                with (
                    tc.tile_pool(name="phB", bufs=GBUF) as pB,
                    tc.tile_pool(name="phBs", bufs=int(os.environ.get("KB_SBUFS", "2"))) as pBs,
                    tc.tile_pool(name="phBe", bufs=3) as pBe,
                    tc.tile_pool(name="selfq", bufs=3) as pSq,
                    tc.tile_pool(name="psB", bufs=2, space="PSUM") as psB,
                    tc.tile_pool(name="psBa", bufs=3, space="PSUM") as psBa,
                ):

                    def self1(b):
                        t = pSq.tile([P, IN_C], dt.bfloat16, tag="selft")
                        nc.sync.dma_start(out=t[:], in_=xself[b * P : (b + 1) * P, :])
                        return t[:]

                    def epi1(b, psum):
                        tT = pBe.tile([P, P], dt.bfloat16, tag="tT")
                        nc.vector.tensor_tensor(
                            out=tT[:],
                            in0=psum[:],
                            in1=gconst["dinvrep"][:, b * P : (b + 1) * P],
                            op=OP.mult,
                        )
                        ph1 = psB.tile([P, P], dt.float32, space="PSUM", tag="ph1")
                        nc.tensor.matmul(out=ph1[:], lhsT=gconst["W1"][:], rhs=tT[:],
                                         start=True, stop=True)
                        h1T = pBe.tile([P, P], dt.bfloat16, tag="h1T")
                        nc.scalar.activation(
                            out=h1T[:], in_=ph1[:],
                            func=mybir.ActivationFunctionType.Relu,
                            bias=gconst["b1c"][:, 0:1], scale=1.0,
                        )
                        pu = psB.tile([P, OUT_C], dt.float32, space="PSUM", tag="pu")
                        nc.tensor.matmul(out=pu[:], lhsT=h1T[:], rhs=gconst["W2"][:],
                                         start=True, stop=True)
                        nc.scalar.mul(
                            out=u2panel[:, b * OUT_C : (b + 1) * OUT_C],
                            in_=pu[:],
                            mul=gconst["dinv_blk"][:, b : b + 1],
                        )

                    ccs = [None] * NQ

                    def post1(b):
                        if not (dbg_epi and dbg_agg):
                            return
                        if os.environ.get("KB_CC", "1") == "0":
                            return
                        for j in range(NQ):
                            if b != QB0[j] + QBLK[j] - 1:
                                continue
                            src = u2panel[:, QB0[j] * OUT_C : (QB0[j] + QBLK[j]) * OUT_C]
                            dmas = []
                            for half in range(2):
                                dst = u2locq[j].ap().rearrange(
                                    "(b p) (two h) -> p b two h", p=P, two=2
                                )[:, :, half, :]
                                dmas.append(nc.sync.dma_start(
                                    out=dst,
                                    in_=src.rearrange("p (b h) -> p b h", h=OUT_C),
                                ))
                            cc = nc.gpsimd.collective_compute(
                                "AllGather",
                                mybir.AluOpType.bypass,
                                replica_groups=[list(range(NCORES))],
                                ins=[u2locq[j][:]],
                                outs=[T2q[j][:]],
                            )
                            for d in dmas:
                                _add_dep(cc.ins, d.ins, sync=True,
                                         reason="allgather waits for u2loc write")
                            ccs[j] = cc

                    layer_pass(tc, sched1, tbl1, gidx1_t, dstloc1_t, gconst["dinvrep"],
                               pB, pBs, psBa, pBe, gconst, P, [0] * NQ, epi1,
                               self1, post_block=post1)

                g1cm.__exit__(None, None, None)

                if stop_after == "L1":
                    with tc.tile_pool(name="dbg", bufs=1) as dbg:
                        z = dbg.tile([P, BPC * OUT_C], dt.float32, tag="z")
                        nc.vector.memset(z[:], 0)
                        nc.sync.dma_start(
                            out=y.ap().rearrange("(b p) h -> p b h", p=P),
                            in_=z[:].rearrange("p (b h) -> p b h", h=OUT_C),
                        )

                # ---------------- T2 AllGather ---------------------------------
                g2cm = tc.tile_pool(name="g2", bufs=1)
                g2p = g2cm.__enter__()
                gidx2_t = []
                for q in range(NQ):
                    tl = g2p.tile([P, sched2["slen"][q] // 16], dt.int16, tag=f"gi2_{q}")
                    nc.sync.dma_start(out=tl[:], in_=gidx2[q][:])
                    gidx2_t.append(tl)
                tl = g2p.tile([P, sched2["totcols"]], dt.bfloat16, tag="dl2")
                nc.sync.dma_start(out=tl[:], in_=dstloc2[:])
                dstloc2_t = tl

                run_d = stop_after not in ("L1", "CC")
                if stop_after == "CC":
                    with tc.tile_pool(name="dbgC", bufs=1) as dbg:
                        z = dbg.tile([P, BPC * OUT_C], dt.float32, tag="zC")
                        nc.vector.memset(z[:], 0)
                        nc.sync.dma_start(
                            out=y.ap().rearrange("(b p) h -> p b h", p=P),
                            in_=z[:].rearrange("p (b h) -> p b h", h=OUT_C),
                        )

                # ---------------- L2: gather T2 pairs, aggregate, FC ------------
                if run_d:
                    tbl2 = [T2q[q][:] for q in range(NQ)]
                    rhs_off2 = [0] * NQ
                    with (
                        tc.tile_pool(name="phD", bufs=GBUF) as pD,
                        tc.tile_pool(name="phDs", bufs=int(os.environ.get("KB_SBUFS", "2"))) as pDs,
                        tc.tile_pool(name="phDe", bufs=3) as pDe,
                        tc.tile_pool(name="ypl", bufs=3) as ypool,
                        tc.tile_pool(name="psD", bufs=2, space="PSUM") as psD,
                        tc.tile_pool(name="psDa", bufs=3, space="PSUM") as psDa,
                    ):
                        if not (dbg_epi and dbg_agg):
                            with tc.tile_pool(name="dbgy", bufs=1) as dbgy:
                                z = dbgy.tile([P, BPC * OUT_C], dt.float32, tag="zY")
                                nc.vector.memset(z[:], 0)
                                nc.sync.dma_start(
                                    out=y.ap().rearrange("(b p) h -> p b h", p=P),
                                    in_=z[:].rearrange("p (b h) -> p b h", h=OUT_C),
                                )

                        def epi2(b, psum):
                            tT2 = pDe.tile([OUT_C, P], dt.float32, tag="tT2")
                            nc.vector.tensor_tensor(
                                out=tT2[:],
                                in0=psum[:],
                                in1=gconst["dinvrep"][:OUT_C, b * P : (b + 1) * P],
                                op=OP.mult,
                            )
                            h2T = pDe.tile([OUT_C, P], dt.bfloat16, tag="h2T")
                            nc.scalar.activation(
                                out=h2T[:], in_=tT2[:],
                                func=mybir.ActivationFunctionType.Identity,
                                bias=gconst["b2c"][:, 0:1], scale=1.0,
                            )
                            py = psD.tile([P, OUT_C], dt.float32, space="PSUM", tag="py")
                            nc.tensor.matmul(out=py[:], lhsT=h2T[:], rhs=gconst["Wfc"][:],
                                             start=True, stop=False)
                            nc.tensor.matmul(out=py[:], lhsT=gconst["ones1"][:1, :],
                                             rhs=gconst["bfcr"][:1, :],
                                             start=False, stop=True)
                            yt = ypool.tile([P, OUT_C], dt.float32, tag="yt")
                            nc.scalar.copy(out=yt[:], in_=py[:])
                            nc.sync.dma_start(
                                out=y[b * P : (b + 1) * P, :], in_=yt[:]
                            )

                        layer_pass(tc, sched2, tbl2, gidx2_t, dstloc2_t,
                                   gconst["dinvrep"], pD, pDs, psDa, pDe, gconst,
                                   OUT_C, rhs_off2, epi2,
                                   lambda b: u2panel[:, b * OUT_C : (b + 1) * OUT_C],
                                   gates=ccs)
                g2cm.__exit__(None, None, None)
                u2cm.__exit__(None, None, None)

    nc.compile()
    return nc


def _make_runner(nc):
    """jit-compiled SPMD runner over 8 cores (reusable across calls so
    executions can be timed warm)."""
    import jax
    import numpy as np
    from jax.sharding import Mesh, PartitionSpec
    from jax.experimental.shard_map import shard_map
    import concourse.mybir as mybir
    from concourse import bass2jax

    bass2jax.install_neuronx_cc_hook()
    partition_name = nc.partition_id_tensor.name if nc.partition_id_tensor else None
    in_names, out_names, out_avals, zero_outs = [], [], [], []
    for alloc in nc.m.functions[0].allocations:
        if not isinstance(alloc, mybir.MemoryLocationSet):
            continue
        name = alloc.memorylocations[0].name
        if alloc.kind == "ExternalInput":
            if name != partition_name:
                in_names.append(name)
        elif alloc.kind == "ExternalOutput":
            out_names.append(name)
            shape = tuple(alloc.tensor_shape)
            dtype = mybir.dt.np(alloc.dtype)
            out_avals.append(jax.core.ShapedArray(shape, dtype))
            zero_outs.append(np.zeros(shape, dtype))
    n_params = len(in_names)
    all_in_names = list(in_names) + list(out_names)
    if partition_name is not None:
        all_in_names.append(partition_name)

    def _body(*args):
        operands = list(args)
        if partition_name is not None:
            operands.append(bass2jax.partition_id_tensor())
        outs = bass2jax._bass_exec_p.bind(
            *operands,
            out_avals=tuple(out_avals),
            in_names=tuple(all_in_names),
            out_names=tuple(out_names),
            lowering_input_output_aliases=(),
            sim_require_finite=True,
            sim_require_nnan=True,
            nc=nc,
        )
        return tuple(outs)

    devices = jax.devices()[:NCORES]
    mesh = Mesh(np.asarray(devices), ("core",))
    in_specs = (PartitionSpec("core"),) * (n_params + len(out_names))
    out_specs = (PartitionSpec("core"),) * len(out_names)
    fn = jax.jit(
        shard_map(_body, mesh=mesh, in_specs=in_specs, out_specs=out_specs,
                  check_rep=False),
        keep_unused=True,
    )
    return fn, in_names, out_names, zero_outs, mesh


def kernel(x, edge_index, W1, b1, W2, b2, Wfc, bfc, _trace=False, _bench=True):
    import time as _time
    import jax
    from jax.sharding import NamedSharding, PartitionSpec

    import os as _os
    sched1, sched2, in_maps, perm_pos = _preprocess(
        x, edge_index, W1, b1, W2, b2, Wfc, bfc)
    key = (tuple(sched1["slen"]), tuple(sched2["slen"]),
           _os.environ.get("KB_REPEAT", "1"),
           _os.environ.get("KB_STOP_AFTER", ""),
           _os.environ.get("KB_BARRIERS", "11"),
           _os.environ.get("KB_SCRATCH", ""), _os.environ.get("KB_GBUF", ""),
           _os.environ.get("KB_CALL", ""), _os.environ.get("KB_CC", "1"),
           _os.environ.get("KB_SBUFS", ""), _os.environ.get("KB_SP", "1"),
           _os.environ.get("KB_EPI", "1"), _os.environ.get("KB_AGG", "1"))
    if key not in _kernel_cache:
        nc = _build(sched1, sched2)
        _kernel_cache[key] = (nc, _make_runner(nc))
    nc, (fn, in_names, out_names, zero_outs, mesh) = _kernel_cache[key]

    sh = NamedSharding(mesh, PartitionSpec("core"))
    concat_in = [
        np.concatenate([np.asarray(in_maps[c][nm]) for c in range(NCORES)], axis=0)
        for nm in in_names
    ]
    concat_zeros = [
        np.zeros((NCORES * z.shape[0], *z.shape[1:]), z.dtype) for z in zero_outs
    ]
    dev_in = [jax.device_put(a, sh) for a in concat_in + concat_zeros]
    out_arrs = fn(*dev_in)
    jax.block_until_ready(out_arrs)

    if _bench:
        times = []
        for _ in range(5):
            t0 = _time.perf_counter()
            out_arrs = fn(*dev_in)
            jax.block_until_ready(out_arrs)
            times.append(_time.perf_counter() - t0)
        kernel._last_times = times
        kernel._last_exec_time_ns = int(min(times) * 1e9)
    else:
        kernel._last_exec_time_ns = None
    if not hasattr(kernel, "_runners"):
        kernel._runners = {}
    kernel._runners[_os.environ.get("KB_REPEAT", "1")] = (fn, dev_in)

    outs = {nm: np.asarray(out_arrs[i]) for i, nm in enumerate(out_names)}
    Y = outs["y"].reshape(NCORES * SHARD, OUT_C)
    return Y[perm_pos[:N]].astype(np.float32)


# revision 24
# speedup vs baseline: 1.2160x; 1.2160x over previous
"""Trainium2 Bass kernel for 2-layer GCN (GCNConv -> relu -> GCNConv -> Linear).

Strategy (8 NeuronCores, SPMD), v2:
  - Nodes padded to NPAD=100352 and dealt (serpentine, by degree) into 784
    blocks of 128 slots; 98 blocks per core (edge partition by destination).
  - Layer-1 table is just xd = (x * dinv)[position order] (bf16), uploaded as
    an input: aggregation commutes with the linear transform, so x is
    aggregated first and @W1 happens per destination block afterwards.  No
    on-device table build, no T1 AllGather, no barrier before L1.
  - Aggregation is TRANSPOSED: psum[feat, slot] += gt_chunk.T @ S_chunk where
    S is the 0/1 selection matrix (DVE is_equal) and gt are dma_gather'ed
    message rows.  The dst-side dinv is then a per-block DVE multiply with a
    partition-replicated dinv panel (no PE transpose in the epilogue).
  - Self loops are never gathered: one identity matmul per block on the
    contiguous local tile (xself panel for L1, the SBUF-resident u2panel for
    L2).
  - Gather streams: per (queue, block) cells padded to multiples of 32 with
    cross-core-max lengths (SPMD shares one program); matmul "runs" merge
    contiguous rows within each 128-row gather column (~20 matmuls/block).
    216.5K rows/core/layer vs 250.9K in v1.
  - Layer-2: u2 rows are duplicated to 256B rows (min gather row size) and
    the table is split into 4 BLOCK-RANGE quarters [25,25,25,23].  Each
    quarter is AllGather'ed by its own collective as soon as its last
    block's epilogue lands (explicit cross-engine deps, no barriers), and
    L2 queue j gathers exclusively from quarter j -- so the collectives
    hide under the L1 gather drain and only the last quarter's (~5 MB) is
    exposed.  Queue loads are rebalanced via the quarter sizes.

Perf notes (measured on these cores):
  - dma_gather is descriptor-paced ~8.4 ns per 256B row per SWDGE queue
    (4-queue hard limit); 426K rows/core over both layers -> ~900 us floor,
    measured directly with KB_AGG=0.  Larger dynamic_dma_scratch_size eats
    SBUF and did not help beyond 65536.
  - Deep gather-tile buffering (GBUF=36) absorbs the epilogue/collective
    pipeline jitter; selection-matrix builds are emitted one block ahead so
    the strict-FIFO DVE never blocks the next block's PE matmuls behind a
    stalled psum read.
  - AllGather outputs must be addr_space="Shared" for the fast HBM-HBM path.
  - Removing the collective/gather ordering deps desyncs the mesh
    (NRT_EXEC_UNIT_UNRECOVERABLE): keep the explicit _add_dep_helper edges.
  - Uniform CALL=1024-index gather calls (partial/2048 calls wedged).
"""

import os
import numpy as np
import ml_dtypes

P = 128
NCORES = 8
NQ = 4
IN_C, HID, OUT_C = 128, 128, 64
CALL = int(os.environ.get("KB_CALL", "1024"))


def _set_size(n_nodes, bpc):
    global N, BPC, NBINS, NPAD, SHARD, WIN, PAIRWIN
    N = n_nodes
    BPC = bpc
    NBINS = NCORES * BPC
    NPAD = NBINS * P
    SHARD = BPC * P
    WIN = NPAD // NQ          # L1 gather window (rows)
    PAIRWIN = NPAD // 4       # L2 gather window (pair rows); 2 windows of NPAD//4 pairs
    assert WIN <= 32768 and PAIRWIN <= 32768


_set_size(100000, 98)

QBLK = [25, 25, 25, 23]                    # L2 source-quarter sizes (blocks)
QB0 = [0, 25, 50, 75]

_kernel_cache = {}


def _wrap_idx(st):
    """[C, SLEN] int16 -> [C, 128, SLEN//16] wrapped+replicated."""
    C, SLEN = st.shape
    w = st.reshape(C, SLEN // 16, 16)
    w = np.swapaxes(w, 1, 2)                       # [C, 16, SLEN//16]
    return np.ascontiguousarray(np.tile(w, (1, 8, 1)))


def _build_layer(core, q, blk, idx, dslot):
    """Build gather streams + schedule for one layer.

    core/q/blk/idx/dslot: per-edge arrays (self loops excluded).
    Layout is shared across cores (cross-core max cell lengths); cells are
    packed back-to-back with NO alignment: every matmul consumes a full
    128-row gather column, and rows of the adjacent block are masked to zero
    in the selection matrix via block-parity disambiguation (dstloc value =
    dslot + 128*(block%2), compared against the block's parity window of a
    [P, 256] iota table; pad rows use sentinel 500).

    Returns (sched, gidx, dstloc):
      sched: dict with slen[q], colbase[q], blocks[b] = per-q (c0, c1, need)
      gidx:  list per q of [NCORES, 128, slen//16] int16
      dstloc: [NCORES, 128, TOTCOLS] bf16
    """
    ncell = NCORES * NQ * BPC
    cell = (core * NQ + q) * BPC + blk
    cnt = np.bincount(cell, minlength=ncell).reshape(NCORES, NQ, BPC)
    ulen = cnt.max(axis=0).astype(np.int64)                           # [NQ, BPC]
    start = np.zeros((NQ, BPC), np.int64)
    start[:, 1:] = np.cumsum(ulen, axis=1)[:, :-1]
    qlen = ulen.sum(axis=1)
    slen = (-(-qlen // CALL)) * CALL                                  # [NQ]
    sbase = np.zeros(NQ + 1, np.int64)
    np.cumsum(slen, out=sbase[1:])
    stot = int(sbase[-1])

    # place edges into per-(core, queue) streams
    order = np.argsort(cell, kind="stable")
    ccnt = np.bincount(cell, minlength=ncell)
    cstart = np.zeros(ncell + 1, np.int64)
    np.cumsum(ccnt, out=cstart[1:])
    rank = np.arange(cell.shape[0]) - cstart[cell[order]]
    co, qo, bo = core[order], q[order], blk[order]
    pos = co * stot + sbase[qo] + start[qo, bo] + rank
    idx_flat = np.zeros(NCORES * stot, np.int16)
    dsl_flat = np.full(NCORES * stot, 500.0, np.float32)
    idx_flat[pos] = idx[order].astype(np.int16)
    dsl_flat[pos] = dslot[order] + 128.0 * (bo % 2)

    gidx = []
    for qq in range(NQ):
        st = idx_flat.reshape(NCORES, stot)[:, sbase[qq] : sbase[qq] + slen[qq]]
        gidx.append(_wrap_idx(np.ascontiguousarray(st)))
    colbase = (sbase[:NQ] // P).astype(np.int64)
    totcols = stot // P
    dl = dsl_flat.reshape(NCORES, totcols, P).transpose(0, 2, 1)  # [C, 128, cols]
    dstloc = np.ascontiguousarray(dl).astype(ml_dtypes.bfloat16)

    blocks = []
    for b in range(BPC):
        per_q = []
        for qq in range(NQ):
            o = int(start[qq, b])
            L = int(ulen[qq, b])
            e = o + L
            if L:
                per_q.append((o // P, (e - 1) // P + 1, (e - 1) // CALL))
            else:
                per_q.append((o // P, o // P, -1))
        blocks.append(per_q)
    sched = {
        "slen": [int(s) for s in slen],
        "colbase": [int(c) for c in colbase],
        "totcols": totcols,
        "blocks": blocks,
        "ncalls": [int(s) // CALL for s in slen],
        "cmax": max(
            max((pq[1] - pq[0]) for pq in per_q) for per_q in blocks
        ),
    }
    return sched, gidx, dstloc


def _preprocess(x, edge_index, W1, b1, W2, b2, Wfc, bfc):
    src = np.asarray(edge_index[0], dtype=np.int64)
    dst = np.asarray(edge_index[1], dtype=np.int64)
    deg = (np.bincount(dst, minlength=N) + 1).astype(np.float32)
    dinv = (1.0 / np.sqrt(deg)).astype(np.float32)

    # serpentine deal by degree -> (bin, slot); balances per-block edge counts
    key = np.zeros(NPAD, np.float32)
    key[:N] = deg
    order = np.argsort(-key, kind="stable")
    i = np.arange(NPAD)
    r, c = i // NBINS, i % NBINS
    bins_for_rank = np.where(r % 2 == 0, c, NBINS - 1 - c)
    perm_bin = np.empty(NPAD, np.int64)
    perm_slot = np.empty(NPAD, np.int64)
    perm_bin[order] = bins_for_rank
    perm_slot[order] = r
    perm_pos = perm_bin * P + perm_slot          # node -> position
    pos2node = np.empty(NPAD, np.int64)
    pos2node[perm_pos] = np.arange(NPAD)

    dinv_pad = np.ones(NPAD, np.float32)
    dinv_pad[:N] = dinv
    dinv_pos = dinv_pad[pos2node]                # dinv by position

    # xd table in POSITION order: xd[pos] = x[node] * dinv[node]
    xdn = np.zeros((NPAD, IN_C), np.float32)
    xdn[:N] = np.asarray(x, np.float32) * dinv[:, None]
    xd = xdn[pos2node].astype(ml_dtypes.bfloat16)             # [NPAD, 128]

    ecore = perm_bin[dst] // BPC
    eb = perm_bin[dst] % BPC
    edslot = perm_slot[dst].astype(np.float32)
    spos = perm_pos[src]

    # L1: window = position quarter, idx = position within window
    q1 = spos // WIN
    sched1, gidx1, dstloc1 = _build_layer(ecore, q1, eb, spos % WIN, edslot)
    # L2: queue = source block-range quarter, idx = row in the quarter table
    qb = np.zeros(BPC, np.int64)
    for j in range(1, NQ):
        qb[QB0[j]:] += 1
    c_src = spos // SHARD
    b_src = (spos % SHARD) // P
    s_src = spos % P
    j2 = qb[b_src]
    rows_j = np.asarray([QBLK[j] * P for j in range(NQ)], np.int64)
    idx2 = c_src * rows_j[j2] + (b_src - np.asarray(QB0)[j2]) * P + s_src
    sched2, gidx2, dstloc2 = _build_layer(ecore, j2, eb, idx2, edslot)

    dinv_blk = dinv_pos.reshape(NCORES, BPC, P).transpose(0, 2, 1)  # [C,128,98]
    dinvrep = np.tile(dinv_pos.reshape(NCORES, 1, BPC * P), (1, P, 1)).astype(
        ml_dtypes.bfloat16
    )                                                               # [C,128,BPC*128]

    common = {
        "W1": np.asarray(W1, np.float32).astype(ml_dtypes.bfloat16),
        "W2": np.asarray(W2, np.float32).astype(ml_dtypes.bfloat16),
        "Wfc": np.asarray(Wfc, np.float32).astype(ml_dtypes.bfloat16),
        "b1c": np.asarray(b1, np.float32).reshape(HID, 1),
        "b2c": np.asarray(b2, np.float32).reshape(OUT_C, 1),
        "bfcr": np.asarray(bfc, np.float32).astype(ml_dtypes.bfloat16).reshape(1, OUT_C),
        "ones1": np.ones((1, P), ml_dtypes.bfloat16),
        "identbf": np.eye(P, dtype=ml_dtypes.bfloat16),
        "iota": np.tile(np.arange(2 * P, dtype=np.float32)[None, :], (P, 1)).astype(
            ml_dtypes.bfloat16
        ),
        "xd": xd,
    }
    in_maps = []
    for cc in range(NCORES):
        m = dict(common)
        m["xself"] = np.ascontiguousarray(xd[cc * SHARD : (cc + 1) * SHARD])
        m["dinv_blk"] = np.ascontiguousarray(dinv_blk[cc])
        m["dinvrep"] = np.ascontiguousarray(dinvrep[cc])
        m["dstloc1"] = np.ascontiguousarray(dstloc1[cc])
        m["dstloc2"] = np.ascontiguousarray(dstloc2[cc])
        for qq in range(NQ):
            m[f"gidx1_{qq}"] = np.ascontiguousarray(gidx1[qq][cc])
            m[f"gidx2_{qq}"] = np.ascontiguousarray(gidx2[qq][cc])
        in_maps.append(m)
    return sched1, sched2, in_maps, perm_pos


def _build(sched1, sched2):
    import concourse.bass as bass  # noqa: F401
    import concourse.mybir as mybir
    import concourse.tile as tile
    from concourse import bacc
    from concourse.bass import _add_dep_helper as _add_dep

    stop_after = os.environ.get("KB_STOP_AFTER", "")   # "", "L1", "CC"
    bar_mask = os.environ.get("KB_BARRIERS", "11")
    bar_mask = "11" if bar_mask == "1" else ("00" if bar_mask == "0" else bar_mask)
    dbg_epi = os.environ.get("KB_EPI", "1") == "1"
    dbg_agg = os.environ.get("KB_AGG", "1") == "1"
    GBUF = int(os.environ.get("KB_GBUF", "36"))
    dt = mybir.dt
    OP = mybir.AluOpType

    nc = bacc.Bacc("TRN2", num_devices=NCORES, target_bir_lowering=False, debug=False,
                   num_swdge_queues=4,
                   dynamic_dma_scratch_size=int(os.environ.get("KB_SCRATCH", "65536")))

    xd = nc.dram_tensor("xd", [NPAD, IN_C], dt.bfloat16, kind="ExternalInput")
    xself = nc.dram_tensor("xself", [SHARD, IN_C], dt.bfloat16, kind="ExternalInput")
    W1 = nc.dram_tensor("W1", [IN_C, HID], dt.bfloat16, kind="ExternalInput")
    W2 = nc.dram_tensor("W2", [HID, OUT_C], dt.bfloat16, kind="ExternalInput")
    Wfc = nc.dram_tensor("Wfc", [OUT_C, OUT_C], dt.bfloat16, kind="ExternalInput")
    b1c = nc.dram_tensor("b1c", [HID, 1], dt.float32, kind="ExternalInput")
    b2c = nc.dram_tensor("b2c", [OUT_C, 1], dt.float32, kind="ExternalInput")
    bfcr = nc.dram_tensor("bfcr", [1, OUT_C], dt.bfloat16, kind="ExternalInput")
    ones1 = nc.dram_tensor("ones1", [1, P], dt.bfloat16, kind="ExternalInput")
    identbf = nc.dram_tensor("identbf", [P, P], dt.bfloat16, kind="ExternalInput")
    iota = nc.dram_tensor("iota", [P, 2 * P], dt.bfloat16, kind="ExternalInput")
    dinv_blk = nc.dram_tensor("dinv_blk", [P, BPC], dt.float32, kind="ExternalInput")
    dinvrep = nc.dram_tensor("dinvrep", [P, BPC * P], dt.bfloat16, kind="ExternalInput")
    dstloc1 = nc.dram_tensor("dstloc1", [P, sched1["totcols"]], dt.bfloat16, kind="ExternalInput")
    dstloc2 = nc.dram_tensor("dstloc2", [P, sched2["totcols"]], dt.bfloat16, kind="ExternalInput")
    gidx1 = [nc.dram_tensor(f"gidx1_{q}", [P, sched1["slen"][q] // 16], dt.int16,
                            kind="ExternalInput") for q in range(NQ)]
    gidx2 = [nc.dram_tensor(f"gidx2_{q}", [P, sched2["slen"][q] // 16], dt.int16,
                            kind="ExternalInput") for q in range(NQ)]
    y = nc.dram_tensor("y", [SHARD, OUT_C], dt.float32, kind="ExternalOutput")

    u2locq = [nc.dram_tensor(f"u2loc_{j}", [QBLK[j] * P, 2 * OUT_C], dt.bfloat16)
              for j in range(NQ)]
    T2q = [nc.dram_tensor(f"T2_{j}", [NCORES * QBLK[j] * P, 2 * OUT_C], dt.bfloat16,
                          addr_space="Shared") for j in range(NQ)]

    def layer_pass(tc, sched, tables, gidx_t, dstloc_t, dinvrep_t, gp, sp, ps, pse,
                   gconst, nfeat, rhs_off, epilogue, self_lhsT,
                   gates=None, post_block=None):
        """One aggregation layer: gathers + selection matmuls + epilogues.

        tables: per-q in_ap for dma_gather; rhs_off: per-q feature offset into
        the gathered 128-elem row; nfeat: features per message; self_lhsT(b):
        [128, nfeat] SBUF AP of the block's self rows; epilogue(b, psum) with
        psum [nfeat, 128] = aggregated messages transposed (no dst dinv yet).
        gates[q]: instruction each queue-q gather must wait for (collective);
        post_block(b): hook after block b's epilogue (emits u2 DMAs+collectives).
        """
        issued = [0] * NQ
        gtiles = {}

        def issue(q):
            call = issued[q]
            gt = gp.tile([P, CALL // P, P], dt.bfloat16, tag="gbuf")
            g = nc.gpsimd.dma_gather(
                out_ap=gt[:],
                in_ap=tables[q],
                idxs_ap=gidx_t[q][:, call * (CALL // 16) : (call + 1) * (CALL // 16)],
                num_idxs=CALL,
                num_idxs_reg=CALL,
                elem_size=P,
                queue_num=q,
                single_packet=os.environ.get("KB_SP", "1") == "1",
            )
            if gates is not None and gates[q] is not None:
                _add_dep(g.ins, gates[q].ins, sync=True,
                         reason="gather waits for quarter allgather")
            gtiles[(q, call)] = gt
            issued[q] = call + 1

        cmax = sched["cmax"]

        def build_s(b):
            per_q = sched["blocks"][b]
            par = b % 2
            stiles = []
            for q in range(NQ):
                c0, c1, _ = per_q[q]
                st = sp.tile([P, cmax, P], dt.bfloat16, tag=f"s{q}")
                if c1 > c0:
                    cb = sched["colbase"][q]
                    nc.vector.tensor_tensor(
                        out=st[:, : c1 - c0, :],
                        in0=dstloc_t[:, cb + c0 : cb + c1].to_broadcast([P, c1 - c0, P]),
                        in1=gconst["iota"][:, par * P : (par + 1) * P]
                        .rearrange("p (a b) -> p a b", a=1)
                        .to_broadcast([P, c1 - c0, P]),
                        op=OP.is_equal,
                    )
                stiles.append(st)
            return stiles

        stiles_next = build_s(0) if dbg_agg else None
        for b in range(BPC):
            per_q = sched["blocks"][b]
            for q in range(NQ):
                need = per_q[q][2]
                while issued[q] <= need and issued[q] < sched["ncalls"][q]:
                    issue(q)
            if not dbg_agg:
                continue
            stiles = stiles_next
            psum = ps.tile([nfeat, P], dt.float32, space="PSUM", tag="aggT")
            nc.tensor.matmul(out=psum[:], lhsT=self_lhsT(b), rhs=gconst["identbf"][:],
                             start=True, stop=False)
            nruns = sum(pq[1] - pq[0] for pq in per_q)
            k = 0
            for q in range(NQ):
                c0, c1, _ = per_q[q]
                for col in range(c0, c1):
                    gt = gtiles[(q, col // (CALL // P))]
                    k += 1
                    nc.tensor.matmul(
                        out=psum[:],
                        lhsT=gt[:, col % (CALL // P),
                                rhs_off[q] : rhs_off[q] + nfeat],
                        rhs=stiles[q][:, col - c0, :],
                        start=False,
                        stop=(k == nruns),
                    )
            if b + 1 < BPC:
                stiles_next = build_s(b + 1)
            if dbg_epi:
                epilogue(b, psum)
            else:
                junk = pse.tile([nfeat, P], dt.float32, tag="junk")
                nc.scalar.copy(out=junk[:], in_=psum[:])
            if post_block is not None:
                post_block(b)

    with tile.TileContext(nc) as tc:
        with tc.tile_pool(name="const", bufs=1) as cp:
            gconst = {}
            for name, t, shape, dtt in [
                ("W1", W1, [IN_C, HID], dt.bfloat16),
                ("W2", W2, [HID, OUT_C], dt.bfloat16),
                ("Wfc", Wfc, [OUT_C, OUT_C], dt.bfloat16),
                ("b1c", b1c, [HID, 1], dt.float32),
                ("b2c", b2c, [OUT_C, 1], dt.float32),
                ("bfcr", bfcr, [1, OUT_C], dt.bfloat16),
                ("ones1", ones1, [1, P], dt.bfloat16),
                ("identbf", identbf, [P, P], dt.bfloat16),
                ("iota", iota, [P, 2 * P], dt.bfloat16),
                ("dinv_blk", dinv_blk, [P, BPC], dt.float32),
                ("dinvrep", dinvrep, [P, BPC * P], dt.bfloat16),
            ]:
                tl = cp.tile(shape, dtt, tag=name)
                nc.sync.dma_start(out=tl[:], in_=t[:])
                gconst[name] = tl

            for _rep in range(int(os.environ.get("KB_REPEAT", "1"))):
                if _rep > 0:
                    tc.strict_bb_all_engine_barrier()
                # ---------------- L1: gather xd, aggregate, u2 ------------------
                u2cm = tc.tile_pool(name="u2p", bufs=1)
                u2pool = u2cm.__enter__()
                u2panel = u2pool.tile([P, BPC * OUT_C], dt.bfloat16, tag="u2panel")
                g1cm = tc.tile_pool(name="g1", bufs=1)
                g1p = g1cm.__enter__()
                gidx1_t = []
                for q in range(NQ):
                    tl = g1p.tile([P, sched1["slen"][q] // 16], dt.int16, tag=f"gi1_{q}")
                    nc.sync.dma_start(out=tl[:], in_=gidx1[q][:])
                    gidx1_t.append(tl)
                tl = g1p.tile([P, sched1["totcols"]], dt.bfloat16, tag="dl1")
                nc.sync.dma_start(out=tl[:], in_=dstloc1[:])
                dstloc1_t = tl
                if not (dbg_epi and dbg_agg):
                    nc.vector.memset(u2panel[:], 0)

                tbl1 = [xd[q * WIN : (q + 1) * WIN, :] for q in range(NQ)]

                with (
                    tc.tile_pool(name="phB", bufs=GBUF) as pB,
                    tc.tile_pool(name="phBs", bufs=int(os.environ.get("KB_SBUFS", "2"))) as pBs,
                    tc.tile_pool(name="phBe", bufs=3) as pBe,
                    tc.tile_pool(name="selfq", bufs=3) as pSq,
                    tc.tile_pool(name="psB", bufs=2, space="PSUM") as psB,
                    tc.tile_pool(name="psBa", bufs=int(os.environ.get("KB_PSA", "4")), space="PSUM") as psBa,
                ):

                    def self1(b):
                        t = pSq.tile([P, IN_C], dt.bfloat16, tag="selft")
                        nc.sync.dma_start(out=t[:], in_=xself[b * P : (b + 1) * P, :])
                        return t[:]

                    def epi1(b, psum):
                        tT = pBe.tile([P, P], dt.bfloat16, tag="tT")
                        nc.vector.tensor_tensor(
                            out=tT[:],
                            in0=psum[:],
                            in1=gconst["dinvrep"][:, b * P : (b + 1) * P],
                            op=OP.mult,
                        )
                        ph1 = psB.tile([P, P], dt.float32, space="PSUM", tag="ph1")
                        nc.tensor.matmul(out=ph1[:], lhsT=gconst["W1"][:], rhs=tT[:],
                                         start=True, stop=True)
                        h1T = pBe.tile([P, P], dt.bfloat16, tag="h1T")
                        nc.scalar.activation(
                            out=h1T[:], in_=ph1[:],
                            func=mybir.ActivationFunctionType.Relu,
                            bias=gconst["b1c"][:, 0:1], scale=1.0,
                        )
                        pu = psB.tile([P, OUT_C], dt.float32, space="PSUM", tag="pu")
                        nc.tensor.matmul(out=pu[:], lhsT=h1T[:], rhs=gconst["W2"][:],
                                         start=True, stop=True)
                        nc.scalar.mul(
                            out=u2panel[:, b * OUT_C : (b + 1) * OUT_C],
                            in_=pu[:],
                            mul=gconst["dinv_blk"][:, b : b + 1],
                        )

                    ccs = [None] * NQ

                    def post1(b):
                        if not (dbg_epi and dbg_agg):
                            return
                        if os.environ.get("KB_CC", "1") == "0":
                            return
                        for j in range(NQ):
                            if b != QB0[j] + QBLK[j] - 1:
                                continue
                            src = u2panel[:, QB0[j] * OUT_C : (QB0[j] + QBLK[j]) * OUT_C]
                            dmas = []
                            for half in range(2):
                                dst = u2locq[j].ap().rearrange(
                                    "(b p) (two h) -> p b two h", p=P, two=2
                                )[:, :, half, :]
                                dmas.append(nc.sync.dma_start(
                                    out=dst,
                                    in_=src.rearrange("p (b h) -> p b h", h=OUT_C),
                                ))
                            cc = nc.gpsimd.collective_compute(
                                "AllGather",
                                mybir.AluOpType.bypass,
                                replica_groups=[list(range(NCORES))],
                                ins=[u2locq[j][:]],
                                outs=[T2q[j][:]],
                            )
                            for d in dmas:
                                _add_dep(cc.ins, d.ins, sync=True,
                                         reason="allgather waits for u2loc write")
                            ccs[j] = cc

                    layer_pass(tc, sched1, tbl1, gidx1_t, dstloc1_t, gconst["dinvrep"],
                               pB, pBs, psBa, pBe, gconst, P, [0] * NQ, epi1,
                               self1, post_block=post1)

                g1cm.__exit__(None, None, None)

                if stop_after == "L1":
                    with tc.tile_pool(name="dbg", bufs=1) as dbg:
                        z = dbg.tile([P, BPC * OUT_C], dt.float32, tag="z")
                        nc.vector.memset(z[:], 0)
                        nc.sync.dma_start(
                            out=y.ap().rearrange("(b p) h -> p b h", p=P),
                            in_=z[:].rearrange("p (b h) -> p b h", h=OUT_C),
                        )

                # ---------------- T2 AllGather ---------------------------------
                g2cm = tc.tile_pool(name="g2", bufs=1)
                g2p = g2cm.__enter__()
                gidx2_t = []
                for q in range(NQ):
                    tl = g2p.tile([P, sched2["slen"][q] // 16], dt.int16, tag=f"gi2_{q}")
                    nc.sync.dma_start(out=tl[:], in_=gidx2[q][:])
                    gidx2_t.append(tl)
                tl = g2p.tile([P, sched2["totcols"]], dt.bfloat16, tag="dl2")
                nc.sync.dma_start(out=tl[:], in_=dstloc2[:])
                dstloc2_t = tl

                run_d = stop_after not in ("L1", "CC")
                if stop_after == "CC":
                    with tc.tile_pool(name="dbgC", bufs=1) as dbg:
                        z = dbg.tile([P, BPC * OUT_C], dt.float32, tag="zC")
                        nc.vector.memset(z[:], 0)
                        nc.sync.dma_start(
                            out=y.ap().rearrange("(b p) h -> p b h", p=P),
                            in_=z[:].rearrange("p (b h) -> p b h", h=OUT_C),
                        )

                # ---------------- L2: gather T2 pairs, aggregate, FC ------------
                if run_d:
                    tbl2 = [T2q[q][:] for q in range(NQ)]
                    rhs_off2 = [0] * NQ
                    with (
                        tc.tile_pool(name="phD", bufs=GBUF) as pD,
                        tc.tile_pool(name="phDs", bufs=int(os.environ.get("KB_SBUFS", "2"))) as pDs,
                        tc.tile_pool(name="phDe", bufs=3) as pDe,
                        tc.tile_pool(name="ypl", bufs=3) as ypool,
                        tc.tile_pool(name="psD", bufs=2, space="PSUM") as psD,
                        tc.tile_pool(name="psDa", bufs=int(os.environ.get("KB_PSA", "4")), space="PSUM") as psDa,
                    ):
                        if not (dbg_epi and dbg_agg):
                            with tc.tile_pool(name="dbgy", bufs=1) as dbgy:
                                z = dbgy.tile([P, BPC * OUT_C], dt.float32, tag="zY")
                                nc.vector.memset(z[:], 0)
                                nc.sync.dma_start(
                                    out=y.ap().rearrange("(b p) h -> p b h", p=P),
                                    in_=z[:].rearrange("p (b h) -> p b h", h=OUT_C),
                                )

                        def epi2(b, psum):
                            tT2 = pDe.tile([OUT_C, P], dt.float32, tag="tT2")
                            nc.vector.tensor_tensor(
                                out=tT2[:],
                                in0=psum[:],
                                in1=gconst["dinvrep"][:OUT_C, b * P : (b + 1) * P],
                                op=OP.mult,
                            )
                            h2T = pDe.tile([OUT_C, P], dt.bfloat16, tag="h2T")
                            nc.scalar.activation(
                                out=h2T[:], in_=tT2[:],
                                func=mybir.ActivationFunctionType.Identity,
                                bias=gconst["b2c"][:, 0:1], scale=1.0,
                            )
                            py = psD.tile([P, OUT_C], dt.float32, space="PSUM", tag="py")
                            nc.tensor.matmul(out=py[:], lhsT=h2T[:], rhs=gconst["Wfc"][:],
                                             start=True, stop=False)
                            nc.tensor.matmul(out=py[:], lhsT=gconst["ones1"][:1, :],
                                             rhs=gconst["bfcr"][:1, :],
                                             start=False, stop=True)
                            yt = ypool.tile([P, OUT_C], dt.float32, tag="yt")
                            nc.scalar.copy(out=yt[:], in_=py[:])
                            nc.sync.dma_start(
                                out=y[b * P : (b + 1) * P, :], in_=yt[:]
                            )

                        layer_pass(tc, sched2, tbl2, gidx2_t, dstloc2_t,
                                   gconst["dinvrep"], pD, pDs, psDa, pDe, gconst,
                                   OUT_C, rhs_off2, epi2,
                                   lambda b: u2panel[:, b * OUT_C : (b + 1) * OUT_C],
                                   gates=ccs)
                g2cm.__exit__(None, None, None)
                u2cm.__exit__(None, None, None)

    nc.compile()
    return nc


def _make_runner(nc):
    """jit-compiled SPMD runner over 8 cores (reusable across calls so
    executions can be timed warm)."""
    import jax
    import numpy as np
    from jax.sharding import Mesh, PartitionSpec
    from jax.experimental.shard_map import shard_map
    import concourse.mybir as mybir
    from concourse import bass2jax

    bass2jax.install_neuronx_cc_hook()
    partition_name = nc.partition_id_tensor.name if nc.partition_id_tensor else None
    in_names, out_names, out_avals, zero_outs = [], [], [], []
    for alloc in nc.m.functions[0].allocations:
        if not isinstance(alloc, mybir.MemoryLocationSet):
            continue
        name = alloc.memorylocations[0].name
        if alloc.kind == "ExternalInput":
            if name != partition_name:
                in_names.append(name)
        elif alloc.kind == "ExternalOutput":
            out_names.append(name)
            shape = tuple(alloc.tensor_shape)
            dtype = mybir.dt.np(alloc.dtype)
            out_avals.append(jax.core.ShapedArray(shape, dtype))
            zero_outs.append(np.zeros(shape, dtype))
    n_params = len(in_names)
    all_in_names = list(in_names) + list(out_names)
    if partition_name is not None:
        all_in_names.append(partition_name)

    def _body(*args):
        operands = list(args)
        if partition_name is not None:
            operands.append(bass2jax.partition_id_tensor())
        outs = bass2jax._bass_exec_p.bind(
            *operands,
            out_avals=tuple(out_avals),
            in_names=tuple(all_in_names),
            out_names=tuple(out_names),
            lowering_input_output_aliases=(),
            sim_require_finite=True,
            sim_require_nnan=True,
            nc=nc,
        )
        return tuple(outs)

    devices = jax.devices()[:NCORES]
    mesh = Mesh(np.asarray(devices), ("core",))
    in_specs = (PartitionSpec("core"),) * (n_params + len(out_names))
    out_specs = (PartitionSpec("core"),) * len(out_names)
    fn = jax.jit(
        shard_map(_body, mesh=mesh, in_specs=in_specs, out_specs=out_specs,
                  check_rep=False),
        keep_unused=True,
    )
    return fn, in_names, out_names, zero_outs, mesh


def kernel(x, edge_index, W1, b1, W2, b2, Wfc, bfc, _trace=False, _bench=True):
    import time as _time
    import jax
    from jax.sharding import NamedSharding, PartitionSpec

    import os as _os
    sched1, sched2, in_maps, perm_pos = _preprocess(
        x, edge_index, W1, b1, W2, b2, Wfc, bfc)
    key = (tuple(sched1["slen"]), tuple(sched2["slen"]),
           _os.environ.get("KB_REPEAT", "1"),
           _os.environ.get("KB_STOP_AFTER", ""),
           _os.environ.get("KB_BARRIERS", "11"),
           _os.environ.get("KB_SCRATCH", ""), _os.environ.get("KB_GBUF", ""),
           _os.environ.get("KB_CALL", ""), _os.environ.get("KB_CC", "1"),
           _os.environ.get("KB_SBUFS", ""), _os.environ.get("KB_SP", "1"),
           _os.environ.get("KB_PSA", "4"),
           _os.environ.get("KB_EPI", "1"), _os.environ.get("KB_AGG", "1"))
    if key not in _kernel_cache:
        nc = _build(sched1, sched2)
        _kernel_cache[key] = (nc, _make_runner(nc))
    nc, (fn, in_names, out_names, zero_outs, mesh) = _kernel_cache[key]

    sh = NamedSharding(mesh, PartitionSpec("core"))
    concat_in = [
        np.concatenate([np.asarray(in_maps[c][nm]) for c in range(NCORES)], axis=0)
        for nm in in_names
    ]
    concat_zeros = [
        np.zeros((NCORES * z.shape[0], *z.shape[1:]), z.dtype) for z in zero_outs
    ]
    dev_in = [jax.device_put(a, sh) for a in concat_in + concat_zeros]
    out_arrs = fn(*dev_in)
    jax.block_until_ready(out_arrs)

    if _bench:
        times = []
        for _ in range(5):
            t0 = _time.perf_counter()
            out_arrs = fn(*dev_in)
            jax.block_until_ready(out_arrs)
            times.append(_time.perf_counter() - t0)
        kernel._last_times = times
        kernel._last_exec_time_ns = int(min(times) * 1e9)
    else:
        kernel._last_exec_time_ns = None
    if not hasattr(kernel, "_runners"):
        kernel._runners = {}
    kernel._runners[_os.environ.get("KB_REPEAT", "1")] = (fn, dev_in)

    outs = {nm: np.asarray(out_arrs[i]) for i, nm in enumerate(out_names)}
    Y = outs["y"].reshape(NCORES * SHARD, OUT_C)
    return Y[perm_pos[:N]].astype(np.float32)
